# revision 1
# baseline (speedup 1.0000x reference)
"""Trainium2 Bass kernel for nn_EncoderMemNN_14929306321427 (MemNN encoder).

Math (see reference.py): story (M=256, B=16, S=64) token ids; C (4, V, 128)
embedding tables. Per hop h: m_A = sum_S C[h][s], prob = softmax_M(m_A @ u),
m_C = sum_S C[h+1][s], u += prob @ m_C. u starts at 0, so hop-0's softmax is
uniform: C[0] is never needed and u after hop 0 is mean_M(E1).

Strategy: data-parallel over batch (2 rows/core, 8 cores, no collectives).
Host fuses tables 1..3 into ccat[V+1, 384] fp16 (row V = 0) so each token is
ONE 768B dma_gather row. dma_gather indices are int16, so tokens are split at
32768: call A gathers low tokens from the table base, call B gathers high
tokens from a +32768 row view; slots not owned by a call point at an all-zero
row (PAD row 0 / appended row V), which adds 0 to the sum. Tokens are sorted
within each sentence and sentences are nlow-balanced across groups so the two
calls cover disjoint near-minimal slot ranges (~6% filler). The sentence-sum
runs on the PE as identity-matmul accumulation into PSUM (fp32-exact), then a
tiny PE/ACT/DVE attention pipeline computes the 3 hops.
"""

import numpy as np

HOPS = 3
V = 50257
D = 128
M = 256
B = 16
S = 64
NCORES = 8
BL = B // NCORES            # batch rows per core
NS = BL * M                 # sentences per core
P = 128
NG = NS // P                # sentence groups of 128
DCAT = HOPS * D             # 384 = fused row [C1|C2|C3]
NEG = -1e30
VSPLIT = 32768
ZHIGH = V - VSPLIT          # index of appended zero row within the high view

_CACHE = {}


def _consts():
    ident = np.eye(P, dtype=np.float32)
    i2 = np.eye(2, dtype=np.float32)
    identg = np.eye(P, dtype=np.float16)
    return {"ident": ident, "i2": i2, "identg": identg}


def build(KA, KB, do_compile=True):
    """KA/KB: per-group slot counts for the low/high gather calls."""
    from concourse import bacc, mybir, tile

    f32 = mybir.dt.float32
    f16 = mybir.dt.float16
    i16 = mybir.dt.int16
    Alu = mybir.AluOpType
    Act = mybir.ActivationFunctionType
    Ax = mybir.AxisListType

    nc = bacc.Bacc(num_swdge_queues=2)
    ccat_d = nc.declare_dram_parameter("ccat", [V + 1, DCAT], f16, isOutput=False)
    idx_d = {}
    for g in range(NG):
        idx_d[g, "a"] = nc.declare_dram_parameter(
            f"idxa{g}", [P, P * KA[g] // 16], i16, isOutput=False)
        idx_d[g, "b"] = nc.declare_dram_parameter(
            f"idxb{g}", [P, P * KB[g] // 16], i16, isOutput=False)
    ident_d = nc.declare_dram_parameter("ident", [P, P], f32, isOutput=False)
    identg_d = nc.declare_dram_parameter("identg", [P, P], f16, isOutput=False)
    i2_d = nc.declare_dram_parameter("i2", [2, 2], f32, isOutput=False)
    sel_d = nc.declare_dram_parameter("sel", [P, NG * 2], f32, isOutput=False)
    mneg_d = nc.declare_dram_parameter("mneg", [BL, BL * M], f32, isOutput=False)
    out_d = nc.declare_dram_parameter("out", [BL, D], f32, isOutput=True)

    with tile.TileContext(nc) as tc:
        with (
            tc.tile_pool(name="const", bufs=1) as cpool,
            tc.tile_pool(name="gather", bufs=2) as gpool,
            tc.tile_pool(name="work", bufs=2) as wpool,
            tc.tile_pool(name="ps_e", bufs=2, space="PSUM") as ps_e,
            tc.tile_pool(name="ps_t", bufs=2, space="PSUM") as ps_t,
            tc.tile_pool(name="ps_col", bufs=1, space="PSUM") as ps_col,
            tc.tile_pool(name="ps_mm", bufs=1, space="PSUM") as ps_mm,
        ):
            idx_sb = {}
            for g in range(NG):
                for h in ("a", "b"):
                    t = cpool.tile(list(idx_d[g, h].shape), i16, tag=f"idx{h}{g}")
                    nc.sync.dma_start(out=t[:], in_=idx_d[g, h][:])
                    idx_sb[g, h] = t
            ident = cpool.tile([P, P], f32)
            nc.sync.dma_start(out=ident[:], in_=ident_d[:])
            identg = cpool.tile([P, P], f16)
            nc.sync.dma_start(out=identg[:], in_=identg_d[:])
            i2 = cpool.tile([2, 2], f32)
            nc.sync.dma_start(out=i2[:], in_=i2_d[:])
            sel = cpool.tile([P, NG * 2], f32)
            nc.sync.dma_start(out=sel[:], in_=sel_d[:])
            mneg = cpool.tile([BL, BL * M], f32)
            nc.sync.dma_start(out=mneg[:], in_=mneg_d[:])

            # ---- gather + sentence-sum: E_all[p, g*DCAT+d] = sum_S ccat[tok]
            E_all = cpool.tile([P, NG * DCAT], f32)
            for g in range(NG):
                gta = gpool.tile([P, KA[g], DCAT], f16, tag="gta")
                nc.gpsimd.dma_gather(
                    out_ap=gta[:], in_ap=ccat_d[:], idxs_ap=idx_sb[g, "a"][:],
                    num_idxs=P * KA[g], num_idxs_reg=P * KA[g],
                    elem_size=DCAT, single_packet=False,
                )
                gtb = gpool.tile([P, KB[g], DCAT], f16, tag="gtb")
                nc.gpsimd.dma_gather(
                    out_ap=gtb[:], in_ap=ccat_d[VSPLIT:, :], idxs_ap=idx_sb[g, "b"][:],
                    num_idxs=P * KB[g], num_idxs_reg=P * KB[g],
                    elem_size=DCAT, single_packet=False, queue_num=1,
                )
                eps = ps_e.tile([P, DCAT], f32, tag="eacc")
                tot = KA[g] + KB[g]
                nmm = 0
                for gt, kk in ((gta, KA[g]), (gtb, KB[g])):
                    for r in range(kk):
                        nc.tensor.matmul(
                            out=eps[:], lhsT=identg[:], rhs=gt[:, r, :],
                            start=(nmm == 0), stop=(nmm == tot - 1),
                        )
                        nmm += 1
                nc.vector.tensor_copy(out=E_all[:, g * DCAT:(g + 1) * DCAT], in_=eps[:])

            # transposed E1/E2 for the logits matmuls (filled per group):
            # F_t[:, g*P:(g+1)*P] = (E_t block of group g).T   [d, sentence]
            F1 = cpool.tile([P, NS], f32)
            F2 = cpool.tile([P, NS], f32)
            us = ps_mm.tile([BL, DCAT], f32, tag="usum")
            for g in range(NG):
                for t, F in ((0, F1), (1, F2)):
                    tp = ps_t.tile([P, P], f32, tag="tp")
                    nc.tensor.transpose(
                        out=tp[:],
                        in_=E_all[:, g * DCAT + t * D: g * DCAT + t * D + D],
                        identity=ident[:],
                    )
                    nc.scalar.copy(out=F[:, g * P:(g + 1) * P], in_=tp[:])
                # hop 0: u = mean_M E1[b] (softmax of zero logits is uniform);
                # sel col b marks this group's sentences owned by batch row b
                nc.tensor.matmul(
                    out=us[:], lhsT=sel[:, g * 2:(g + 1) * 2],
                    rhs=E_all[:, g * DCAT:(g + 1) * DCAT],
                    start=(g == 0), stop=(g == NG - 1),
                )
            u = wpool.tile([BL, D], f32, tag="u0")
            nc.scalar.activation(
                out=u[:], in_=us[0:BL, 0:D], func=Act.Copy, scale=1.0 / M
            )

            # ---- hops 1..2
            for hop in (1, 2):
                F = F1 if hop == 1 else F2
                # u as columns: uc[d, b] = u[b, d]
                uc_ps = ps_col.tile([P, BL], f32, tag="colT")
                nc.tensor.matmul(out=uc_ps[:], lhsT=u[:], rhs=i2[:], start=True, stop=True)
                uc = wpool.tile([P, BL], f32, tag="uc")
                nc.scalar.copy(out=uc[:], in_=uc_ps[:])
                # logits[b, b'*M+m] = sum_d u[b,d] * E_hop[b',m,d]; mask kills b!=b'
                lg_ps = ps_mm.tile([BL, NS], f32, tag="lg")
                nc.tensor.matmul(out=lg_ps[:], lhsT=uc[:], rhs=F[:], start=True, stop=True)
                lgm = wpool.tile([BL, NS], f32, tag="lgm")
                nc.vector.scalar_tensor_tensor(
                    out=lgm[:], in0=lg_ps[:], scalar=1.0, in1=mneg[:],
                    op0=Alu.mult, op1=Alu.add,
                )
                nmax = wpool.tile([BL, 1], f32, tag="nmax")
                nc.vector.tensor_reduce(
                    out=nmax[:], in_=lgm[:], axis=Ax.X, op=Alu.max, negate=True
                )
                pe = wpool.tile([BL, NS], f32, tag="pe")
                den = wpool.tile([BL, 1], f32, tag="den")
                nc.scalar.activation(
                    out=pe[:], in_=lgm[:], func=Act.Exp, bias=nmax[:], scale=1.0,
                    accum_out=den[:],
                )
                rden = wpool.tile([BL, 1], f32, tag="rden")
                nc.vector.reciprocal(out=rden[:], in_=den[:])
                # o[b, d] = sum_m pe[b, m] * E_{hop+1}[b, m, d]  (normalized below)
                o_ps = ps_mm.tile([BL, D], f32, tag="o")
                for g in range(NG):
                    pt_ps = ps_col.tile([P, BL], f32, tag="colT")
                    nc.tensor.matmul(
                        out=pt_ps[:], lhsT=pe[:, g * P:(g + 1) * P], rhs=i2[:],
                        start=True, stop=True,
                    )
                    ptsb = wpool.tile([P, BL], f32, tag="ptsb")
                    nc.scalar.copy(out=ptsb[:], in_=pt_ps[:])
                    nc.tensor.matmul(
                        out=o_ps[:], lhsT=ptsb[:],
                        rhs=E_all[:, g * DCAT + hop * D: g * DCAT + hop * D + D],
                        start=(g == 0), stop=(g == NG - 1),
                    )
                # u <- u + o / den
                u2 = wpool.tile([BL, D], f32, tag=f"u{hop}")
                nc.vector.scalar_tensor_tensor(
                    out=u2[:], in0=o_ps[:], scalar=rden[:], in1=u[:],
                    op0=Alu.mult, op1=Alu.add,
                )
                u = u2

            nc.sync.dma_start(out=out_d[:], in_=u[:])
    if do_compile:
        nc.compile()
    return nc


def _wrap16(idx):
    """flat [n] int16 -> SBUF layout [128, n//16]: value i at [i%16, i//16],
    replicated to the 8 16-partition groups the Q7 cores read."""
    n = idx.shape[0]
    w = np.zeros((16, n // 16), np.int16)
    w[np.arange(n) % 16, np.arange(n) // 16] = idx
    return np.tile(w, (8, 1))


def prep_inputs(story, C):
    """Host-side: fused fp16 table, sorted/balanced per-core index layouts."""
    story = np.asarray(story)
    C = np.asarray(C, dtype=np.float32)
    s = story.transpose(1, 0, 2).astype(np.int32)       # (B, M, S)
    ccat = np.zeros((V + 1, DCAT), np.float16)
    ccat[:V] = np.concatenate([C[1], C[2], C[3]], axis=1).astype(np.float16)

    # per core: sort tokens in each sentence (low vocab first) and balance
    # sentences across the NG groups by nlow, mixing batch rows freely; the
    # uploaded sel/mneg tensors encode each sentence's batch-row ownership
    sorted_toks = []                                     # [core][g] -> (P, S)
    nlows = []                                           # [core][g] -> (P,)
    owners = []                                          # [core][g] -> (P,) batch row
    for i in range(NCORES):
        blk = s[i * BL:(i + 1) * BL].reshape(NS, S)      # (NS, S)
        own = np.repeat(np.arange(BL), M)                # (NS,)
        nlow = (blk < VSPLIT).sum(1)
        order = np.argsort(nlow, kind="stable")
        st_c, nl_c, ow_c = [], [], []
        for q in ((0, 3, 1, 2) if NG == 4 else range(NG)):
            pick = order[q * P:(q + 1) * P]
            st_c.append(np.sort(blk[pick], axis=1))
            nl_c.append(nlow[pick])
            ow_c.append(own[pick])
        sorted_toks.append(st_c)
        nlows.append(nl_c)
        owners.append(ow_c)

    KA = tuple(int(max(nlows[i][g].max() for i in range(NCORES))) for g in range(NG))
    KB = tuple(S - int(min(nlows[i][g].min() for i in range(NCORES))) for g in range(NG))

    consts = _consts()
    in_maps = []
    for i in range(NCORES):
        m = {"ccat": ccat, **consts}
        sel = np.zeros((P, NG * 2), np.float32)
        mneg = np.full((BL, BL * M), NEG, np.float32)
        for g in range(NG):
            sel[np.arange(P), g * 2 + owners[i][g]] = 1.0
            mneg[owners[i][g], g * P + np.arange(P)] = 0.0
        m["sel"] = sel
        m["mneg"] = mneg
        for g in range(NG):
            toks = sorted_toks[i][g]                     # (P, S) sorted
            nlow = nlows[i][g]                           # (P,)
            ka, kb = KA[g], KB[g]
            ks = np.arange(ka)[:, None]                  # slot k -> row k*128+p
            low = np.where(ks < nlow[None, :], toks.T[:ka], 0).astype(np.int16)
            m[f"idxa{g}"] = _wrap16(low.reshape(-1))
            k0 = S - kb
            ksb = (k0 + np.arange(kb))[:, None]
            high = np.where(
                ksb >= nlow[None, :],
                toks.T[k0:].astype(np.int64) - VSPLIT,
                ZHIGH,
            ).astype(np.int16)
            m[f"idxb{g}"] = _wrap16(high.reshape(-1))
        in_maps.append(m)
    return in_maps, KA, KB


def run(in_maps, KA, KB, trace=False, **kwargs):
    from concourse.bass_utils import run_bass_kernel_spmd

    key = (KA, KB)
    if key not in _CACHE:
        _CACHE[key] = build(KA, KB)
    nc = _CACHE[key]
    res = run_bass_kernel_spmd(
        nc, in_maps, core_ids=list(range(NCORES)), trace=trace, **kwargs
    )
    out = np.concatenate([r["out"] for r in res.results], axis=0)
    return out, res


def kernel(story, C):
    in_maps, KA, KB = prep_inputs(story, C)
    out, _ = run(in_maps, KA, KB)
    return out.astype(np.float32)



# revision 3
# speedup vs baseline: 1.3201x; 1.3201x over previous
"""Trainium2 Bass kernel for nn_EncoderMemNN_14929306321427 (MemNN encoder).

Math (see reference.py): story (M=256, B=16, S=64) token ids; C (4, V, 128)
embedding tables. Per hop h: m_A = sum_S C[h][s], prob = softmax_M(m_A @ u),
m_C = sum_S C[h+1][s], u += prob @ m_C. u starts at 0, so hop-0's softmax is
uniform: C[0] is never needed and u after hop 0 is mean_M(E1).

Strategy: data-parallel over batch (2 rows/core, 8 cores, no collectives).
Host fuses tables 1..3 into ccat[V+1, 384] fp16 (row V = 0) so each token is
ONE 768B dma_gather row. dma_gather indices are int16, so tokens are split at
32768: call A gathers low tokens from the table base, call B gathers high
tokens from a +32768 row view; slots not owned by a call point at an all-zero
row (PAD row 0 / appended row V), which adds 0 to the sum. Tokens are sorted
within each sentence and sentences are nlow-balanced across groups so the two
calls cover disjoint near-minimal slot ranges (~6% filler). The sentence-sum
runs on the PE as identity-matmul accumulation into PSUM (fp32-exact), then a
tiny PE/ACT/DVE attention pipeline computes the 3 hops.
"""

import numpy as np

HOPS = 3
V = 50257
D = 128
M = 256
B = 16
S = 64
NCORES = 8
BL = B // NCORES            # batch rows per core
NS = BL * M                 # sentences per core
P = 128
NG = NS // P                # sentence groups of 128
DCAT = HOPS * D             # 384 = fused row [C1|C2|C3]
NEG = -1e30
VSPLIT = 32768
ZHIGH = V - VSPLIT          # index of appended zero row within the high view

_CACHE = {}


def _consts():
    ident = np.eye(P, dtype=np.float32)
    i2 = np.eye(2, dtype=np.float32)
    identg = np.eye(P, dtype=np.float16)
    return {"ident": ident, "i2": i2, "identg": identg}


def build(KA, KB, do_compile=True):
    """KA/KB: per-group slot counts for the low/high gather calls."""
    from concourse import bacc, mybir, tile

    f32 = mybir.dt.float32
    f16 = mybir.dt.float16
    i16 = mybir.dt.int16
    Alu = mybir.AluOpType
    Act = mybir.ActivationFunctionType
    Ax = mybir.AxisListType

    nc = bacc.Bacc(num_swdge_queues=4)
    ccat_d = nc.declare_dram_parameter("ccat", [V + 1, DCAT], f16, isOutput=False)
    idx_d = {}
    for g in range(NG):
        idx_d[g, "a"] = nc.declare_dram_parameter(
            f"idxa{g}", [P, P * KA[g] // 16], i16, isOutput=False)
        idx_d[g, "b"] = nc.declare_dram_parameter(
            f"idxb{g}", [P, P * KB[g] // 16], i16, isOutput=False)
    ident_d = nc.declare_dram_parameter("ident", [P, P], f32, isOutput=False)
    identg_d = nc.declare_dram_parameter("identg", [P, P], f16, isOutput=False)
    i2_d = nc.declare_dram_parameter("i2", [2, 2], f32, isOutput=False)
    sel_d = nc.declare_dram_parameter("sel", [P, NG * 2], f32, isOutput=False)
    mneg_d = nc.declare_dram_parameter("mneg", [BL, BL * M], f32, isOutput=False)
    out_d = nc.declare_dram_parameter("out", [BL, D], f32, isOutput=True)

    with tile.TileContext(nc) as tc:
        with (
            tc.tile_pool(name="const", bufs=1) as cpool,
            tc.tile_pool(name="gather", bufs=2) as gpool,
            tc.tile_pool(name="work", bufs=2) as wpool,
            tc.tile_pool(name="ps_e", bufs=2, space="PSUM") as ps_e,
            tc.tile_pool(name="ps_t", bufs=2, space="PSUM") as ps_t,
            tc.tile_pool(name="ps_col", bufs=1, space="PSUM") as ps_col,
            tc.tile_pool(name="ps_mm", bufs=1, space="PSUM") as ps_mm,
        ):
            idx_sb = {}
            for g in range(NG):
                for h in ("a", "b"):
                    t = cpool.tile(list(idx_d[g, h].shape), i16, tag=f"idx{h}{g}")
                    nc.sync.dma_start(out=t[:], in_=idx_d[g, h][:])
                    idx_sb[g, h] = t
            ident = cpool.tile([P, P], f32)
            nc.sync.dma_start(out=ident[:], in_=ident_d[:])
            identg = cpool.tile([P, P], f16)
            nc.sync.dma_start(out=identg[:], in_=identg_d[:])
            i2 = cpool.tile([2, 2], f32)
            nc.sync.dma_start(out=i2[:], in_=i2_d[:])
            sel = cpool.tile([P, NG * 2], f32)
            nc.sync.dma_start(out=sel[:], in_=sel_d[:])
            mneg = cpool.tile([BL, BL * M], f32)
            nc.sync.dma_start(out=mneg[:], in_=mneg_d[:])

            # ---- gather + sentence-sum: E_all[p, g*DCAT+d] = sum_S ccat[tok]
            E_all = cpool.tile([P, NG * DCAT], f32)
            for g in range(NG):
                # split each gather call in half across SWDGE queues — the 4
                # queue contexts generate descriptors concurrently, and
                # descriptor generation (~8 ns/idx/queue) is the critical path
                gta = gpool.tile([P, KA[g], DCAT], f16, tag="gta")
                ah = KA[g] // 2
                for qn, (k0, k1) in ((0, (0, ah)), (2, (ah, KA[g]))):
                    nc.gpsimd.dma_gather(
                        out_ap=gta[:, k0:k1, :], in_ap=ccat_d[:],
                        idxs_ap=idx_sb[g, "a"][:, k0 * 8:k1 * 8],
                        num_idxs=P * (k1 - k0), num_idxs_reg=P * (k1 - k0),
                        elem_size=DCAT, single_packet=False, queue_num=qn,
                    )
                gtb = gpool.tile([P, KB[g], DCAT], f16, tag="gtb")
                bh = KB[g] // 2
                for qn, (k0, k1) in ((1, (0, bh)), (3, (bh, KB[g]))):
                    nc.gpsimd.dma_gather(
                        out_ap=gtb[:, k0:k1, :], in_ap=ccat_d[VSPLIT:, :],
                        idxs_ap=idx_sb[g, "b"][:, k0 * 8:k1 * 8],
                        num_idxs=P * (k1 - k0), num_idxs_reg=P * (k1 - k0),
                        elem_size=DCAT, single_packet=False, queue_num=qn,
                    )
                eps = ps_e.tile([P, DCAT], f32, tag="eacc")
                tot = KA[g] + KB[g]
                nmm = 0
                for gt, kk in ((gta, KA[g]), (gtb, KB[g])):
                    for r in range(kk):
                        nc.tensor.matmul(
                            out=eps[:], lhsT=identg[:], rhs=gt[:, r, :],
                            start=(nmm == 0), stop=(nmm == tot - 1),
                        )
                        nmm += 1
                nc.vector.tensor_copy(out=E_all[:, g * DCAT:(g + 1) * DCAT], in_=eps[:])

            # transposed E1/E2 for the logits matmuls (filled per group):
            # F_t[:, g*P:(g+1)*P] = (E_t block of group g).T   [d, sentence]
            F1 = cpool.tile([P, NS], f32)
            F2 = cpool.tile([P, NS], f32)
            us = ps_mm.tile([BL, DCAT], f32, tag="usum")
            for g in range(NG):
                for t, F in ((0, F1), (1, F2)):
                    tp = ps_t.tile([P, P], f32, tag="tp")
                    nc.tensor.transpose(
                        out=tp[:],
                        in_=E_all[:, g * DCAT + t * D: g * DCAT + t * D + D],
                        identity=ident[:],
                    )
                    nc.scalar.copy(out=F[:, g * P:(g + 1) * P], in_=tp[:])
                # hop 0: u = mean_M E1[b] (softmax of zero logits is uniform);
                # sel col b marks this group's sentences owned by batch row b
                nc.tensor.matmul(
                    out=us[:], lhsT=sel[:, g * 2:(g + 1) * 2],
                    rhs=E_all[:, g * DCAT:(g + 1) * DCAT],
                    start=(g == 0), stop=(g == NG - 1),
                )
            u = wpool.tile([BL, D], f32, tag="u0")
            nc.scalar.activation(
                out=u[:], in_=us[0:BL, 0:D], func=Act.Copy, scale=1.0 / M
            )

            # ---- hops 1..2
            for hop in (1, 2):
                F = F1 if hop == 1 else F2
                # u as columns: uc[d, b] = u[b, d]
                uc_ps = ps_col.tile([P, BL], f32, tag="colT")
                nc.tensor.matmul(out=uc_ps[:], lhsT=u[:], rhs=i2[:], start=True, stop=True)
                uc = wpool.tile([P, BL], f32, tag="uc")
                nc.scalar.copy(out=uc[:], in_=uc_ps[:])
                # logits[b, b'*M+m] = sum_d u[b,d] * E_hop[b',m,d]; mask kills b!=b'
                lg_ps = ps_mm.tile([BL, NS], f32, tag="lg")
                nc.tensor.matmul(out=lg_ps[:], lhsT=uc[:], rhs=F[:], start=True, stop=True)
                lgm = wpool.tile([BL, NS], f32, tag="lgm")
                nc.vector.scalar_tensor_tensor(
                    out=lgm[:], in0=lg_ps[:], scalar=1.0, in1=mneg[:],
                    op0=Alu.mult, op1=Alu.add,
                )
                nmax = wpool.tile([BL, 1], f32, tag="nmax")
                nc.vector.tensor_reduce(
                    out=nmax[:], in_=lgm[:], axis=Ax.X, op=Alu.max, negate=True
                )
                pe = wpool.tile([BL, NS], f32, tag="pe")
                den = wpool.tile([BL, 1], f32, tag="den")
                nc.scalar.activation(
                    out=pe[:], in_=lgm[:], func=Act.Exp, bias=nmax[:], scale=1.0,
                    accum_out=den[:],
                )
                rden = wpool.tile([BL, 1], f32, tag="rden")
                nc.vector.reciprocal(out=rden[:], in_=den[:])
                # o[b, d] = sum_m pe[b, m] * E_{hop+1}[b, m, d]  (normalized below)
                o_ps = ps_mm.tile([BL, D], f32, tag="o")
                for g in range(NG):
                    pt_ps = ps_col.tile([P, BL], f32, tag="colT")
                    nc.tensor.matmul(
                        out=pt_ps[:], lhsT=pe[:, g * P:(g + 1) * P], rhs=i2[:],
                        start=True, stop=True,
                    )
                    ptsb = wpool.tile([P, BL], f32, tag="ptsb")
                    nc.scalar.copy(out=ptsb[:], in_=pt_ps[:])
                    nc.tensor.matmul(
                        out=o_ps[:], lhsT=ptsb[:],
                        rhs=E_all[:, g * DCAT + hop * D: g * DCAT + hop * D + D],
                        start=(g == 0), stop=(g == NG - 1),
                    )
                # u <- u + o / den
                u2 = wpool.tile([BL, D], f32, tag=f"u{hop}")
                nc.vector.scalar_tensor_tensor(
                    out=u2[:], in0=o_ps[:], scalar=rden[:], in1=u[:],
                    op0=Alu.mult, op1=Alu.add,
                )
                u = u2

            nc.sync.dma_start(out=out_d[:], in_=u[:])
    if do_compile:
        nc.compile()
    return nc


def _wrap16(idx):
    """flat [n] int16 -> SBUF layout [128, n//16]: value i at [i%16, i//16],
    replicated to the 8 16-partition groups the Q7 cores read."""
    n = idx.shape[0]
    w = np.zeros((16, n // 16), np.int16)
    w[np.arange(n) % 16, np.arange(n) // 16] = idx
    return np.tile(w, (8, 1))


def prep_inputs(story, C):
    """Host-side: fused fp16 table, sorted/balanced per-core index layouts."""
    story = np.asarray(story)
    C = np.asarray(C, dtype=np.float32)
    s = story.transpose(1, 0, 2).astype(np.int32)       # (B, M, S)
    ccat = np.zeros((V + 1, DCAT), np.float16)
    ccat[:V] = np.concatenate([C[1], C[2], C[3]], axis=1).astype(np.float16)

    # per core: sort tokens in each sentence (low vocab first) and balance
    # sentences across the NG groups by nlow, mixing batch rows freely; the
    # uploaded sel/mneg tensors encode each sentence's batch-row ownership
    sorted_toks = []                                     # [core][g] -> (P, S)
    nlows = []                                           # [core][g] -> (P,)
    owners = []                                          # [core][g] -> (P,) batch row
    for i in range(NCORES):
        blk = s[i * BL:(i + 1) * BL].reshape(NS, S)      # (NS, S)
        own = np.repeat(np.arange(BL), M)                # (NS,)
        nlow = (blk < VSPLIT).sum(1)
        order = np.argsort(nlow, kind="stable")
        st_c, nl_c, ow_c = [], [], []
        for q in ((0, 3, 1, 2) if NG == 4 else range(NG)):
            pick = order[q * P:(q + 1) * P]
            st_c.append(np.sort(blk[pick], axis=1))
            nl_c.append(nlow[pick])
            ow_c.append(own[pick])
        sorted_toks.append(st_c)
        nlows.append(nl_c)
        owners.append(ow_c)

    KA = tuple(int(max(nlows[i][g].max() for i in range(NCORES))) for g in range(NG))
    KB = tuple(S - int(min(nlows[i][g].min() for i in range(NCORES))) for g in range(NG))

    consts = _consts()
    in_maps = []
    for i in range(NCORES):
        m = {"ccat": ccat, **consts}
        sel = np.zeros((P, NG * 2), np.float32)
        mneg = np.full((BL, BL * M), NEG, np.float32)
        for g in range(NG):
            sel[np.arange(P), g * 2 + owners[i][g]] = 1.0
            mneg[owners[i][g], g * P + np.arange(P)] = 0.0
        m["sel"] = sel
        m["mneg"] = mneg
        for g in range(NG):
            toks = sorted_toks[i][g]                     # (P, S) sorted
            nlow = nlows[i][g]                           # (P,)
            ka, kb = KA[g], KB[g]
            ks = np.arange(ka)[:, None]                  # slot k -> row k*128+p
            low = np.where(ks < nlow[None, :], toks.T[:ka], 0).astype(np.int16)
            m[f"idxa{g}"] = _wrap16(low.reshape(-1))
            k0 = S - kb
            ksb = (k0 + np.arange(kb))[:, None]
            high = np.where(
                ksb >= nlow[None, :],
                toks.T[k0:].astype(np.int64) - VSPLIT,
                ZHIGH,
            ).astype(np.int16)
            m[f"idxb{g}"] = _wrap16(high.reshape(-1))
        in_maps.append(m)
    return in_maps, KA, KB


def run(in_maps, KA, KB, trace=False, **kwargs):
    from concourse.bass_utils import run_bass_kernel_spmd

    key = (KA, KB)
    if key not in _CACHE:
        _CACHE[key] = build(KA, KB)
    nc = _CACHE[key]
    res = run_bass_kernel_spmd(
        nc, in_maps, core_ids=list(range(NCORES)), trace=trace, **kwargs
    )
    out = np.concatenate([r["out"] for r in res.results], axis=0)
    return out, res


def kernel(story, C):
    in_maps, KA, KB = prep_inputs(story, C)
    out, _ = run(in_maps, KA, KB)
    return out.astype(np.float32)



# revision 6
# speedup vs baseline: 1.4818x; 1.1225x over previous
"""Trainium2 Bass kernel for nn_EncoderMemNN_14929306321427 (MemNN encoder).

Math (see reference.py): story (M=256, B=16, S=64) token ids; C (4, V, 128)
embedding tables. Per hop h: m_A = sum_S C[h][s], prob = softmax_M(m_A @ u),
m_C = sum_S C[h+1][s], u += prob @ m_C. u starts at 0, so hop-0's softmax is
uniform: C[0] is never needed and u after hop 0 is mean_M(E1).

Strategy: data-parallel over batch (2 rows/core, 8 cores, no collectives).
Host fuses tables 1..3 into ccat[V+1, 512B] rows: [C1 fp16 | C2 int8 | C3
int8] (row V = 0). Each token is ONE 512B dma_gather row. dma_gather indices
are int16, so tokens split between call A (table base, t<32768) and call B
(base row VB=17490, t>=VB); the overlap zone lets the host pick a per-group
split point KA so every sentence contributes EXACTLY KA rows to A and 64-KA
to B (zero-row padding only for rare infeasible sentences). Each group's 64
slots are cut into 4 equal pieces issued on the 4 SWDGE queues -- descriptor
generation (~8 ns/idx/queue, the critical resource) runs 4-way parallel
across the 8 Q7 cores. The sentence-sum runs on the PE as identity-matmul
accumulation into PSUM (2 slots fused per matmul); the int8 sections are
converted+scaled to fp16 on the DVE first. A small PE/ACT/DVE attention
pipeline computes the 3 hops.
"""

import numpy as np

HOPS = 3
V = 50257
D = 128
M = 256
B = 16
S = 64
NCORES = 8
BL = B // NCORES            # batch rows per core
NS = BL * M                 # sentences per core
P = 128
NG = NS // P                # sentence groups of 128
ROWB = 512                  # fused row bytes: 256 (C1 f16) + 128 + 128 (int8)
NEG = -1e30
IMAX = 32768                # call A covers idx 0..32767
VB = V + 1 - IMAX           # 17490: call B covers rows VB..V (V = zero row)
ZB = V - VB                 # 32767: B-call index of the zero row
CLIP = 0.4                  # int8 clip (~4 sigma of N(0, 0.1))
S8 = CLIP / 127.0

_CACHE = {}


def _pieces(KA_g, KB_g):
    """Cut a group's KT slots into 4 near-equal queue pieces; a piece
    straddling the A/B boundary becomes two calls on the same queue. The
    queue-0 call blocks the GpSimd engine until generated, so issue it last."""
    KT = KA_g + KB_g
    sizes = [KT // 4 + (1 if i < KT % 4 else 0) for i in range(4)]
    cuts = np.cumsum([0] + sizes)
    specs = []
    for q in range(4):
        k0, k1 = int(cuts[q]), int(cuts[q + 1])
        if k1 <= KA_g:
            specs.append((q, k0, k1, True))
        elif k0 >= KA_g:
            specs.append((q, k0, k1, False))
        else:
            specs.append((q, k0, KA_g, True))
            specs.append((q, KA_g, k1, False))
    return [s for s in specs if s[0] != 0] + [s for s in specs if s[0] == 0]


def _blob_cols(KA, KB):
    """Column offsets of per-group idx sections + identg inside the blob."""
    offs, c = {}, 0
    for g in range(NG):
        offs[g, "a"] = c
        c += 8 * KA[g]
        offs[g, "b"] = c
        c += 8 * KB[g]
    offs["identg"] = c
    c += D
    return offs, c


def build(KA, KB, do_compile=True):
    from concourse import bacc, mybir, tile

    f32 = mybir.dt.float32
    f16 = mybir.dt.float16
    i16 = mybir.dt.int16
    i8 = mybir.dt.int8
    Alu = mybir.AluOpType
    Act = mybir.ActivationFunctionType
    Ax = mybir.AxisListType

    offs, NB = _blob_cols(KA, KB)

    nc = bacc.Bacc(num_swdge_queues=4)
    ccat_d = nc.declare_dram_parameter("ccat", [V + 1, ROWB], i8, isOutput=False)
    blob_d = nc.declare_dram_parameter("blob", [P, NB], i16, isOutput=False)
    ident_d = nc.declare_dram_parameter("ident", [P, P], f32, isOutput=False)
    sel_d = nc.declare_dram_parameter("sel", [P, NG * 2], f32, isOutput=False)
    i2_d = nc.declare_dram_parameter("i2", [2, 2], f32, isOutput=False)
    mneg_d = nc.declare_dram_parameter("mneg", [BL, BL * M], f32, isOutput=False)
    out_d = nc.declare_dram_parameter("out", [BL, D], f32, isOutput=True)

    with tile.TileContext(nc) as tc:
        with (
            tc.tile_pool(name="const", bufs=1) as cpool,
            tc.tile_pool(name="gather", bufs=2) as gpool,
            tc.tile_pool(name="conv", bufs=2) as vpool,
            tc.tile_pool(name="work", bufs=2) as wpool,
            tc.tile_pool(name="ps_e1", bufs=1, space="PSUM") as ps_e1,
            tc.tile_pool(name="ps_e23", bufs=2, space="PSUM") as ps_e23,
            tc.tile_pool(name="ps_t", bufs=1, space="PSUM") as ps_t,
            tc.tile_pool(name="ps_col", bufs=1, space="PSUM") as ps_col,
            tc.tile_pool(name="ps_mm", bufs=1, space="PSUM") as ps_mm,
        ):
            # idx blob first -- the first gather only waits on this load
            blob = cpool.tile([P, NB], i16)
            nc.sync.dma_start(out=blob[:], in_=blob_d[:])
            identg = blob[:, offs["identg"]:offs["identg"] + D].bitcast(f16)
            ident = cpool.tile([P, P], f32)
            nc.sync.dma_start(out=ident[:], in_=ident_d[:])
            sel = cpool.tile([P, NG * 2], f32)
            nc.sync.dma_start(out=sel[:], in_=sel_d[:])
            i2 = cpool.tile([2, 2], f32)
            nc.sync.dma_start(out=i2[:], in_=i2_d[:])
            mneg = cpool.tile([BL, BL * M], f32)
            nc.sync.dma_start(out=mneg[:], in_=mneg_d[:])

            # E_all[p, g*384+d]: per-sentence sums [E1|E2|E3] for group g
            E_all = cpool.tile([P, NG * 3 * D], f32)
            F1 = cpool.tile([P, NS], f32)
            F2 = cpool.tile([P, NS], f32)
            us = ps_mm.tile([BL, 3 * D], f32, tag="usum")

            for g in range(NG):
                KT = KA[g] + KB[g]
                gt = gpool.tile([P, KT, ROWB], i8, tag="gt")
                for q, k0, k1, is_a in _pieces(KA[g], KB[g]):
                    if is_a:
                        in_ap = ccat_d[:]
                        c0 = offs[g, "a"] + 8 * k0
                        c1 = offs[g, "a"] + 8 * k1
                    else:
                        in_ap = ccat_d[VB:, :]
                        c0 = offs[g, "b"] + 8 * (k0 - KA[g])
                        c1 = offs[g, "b"] + 8 * (k1 - KA[g])
                    nc.gpsimd.dma_gather(
                        out_ap=gt[:, k0:k1, :], in_ap=in_ap,
                        idxs_ap=blob[:, c0:c1],
                        num_idxs=P * (k1 - k0), num_idxs_reg=P * (k1 - k0),
                        elem_size=ROWB, single_packet=False, queue_num=q,
                    )

                # E1: identity-matmul accumulation of the fp16 C1 sections,
                # 2 slots fused per matmul (sections recombined on DVE)
                e1ps = ps_e1.tile([P, 2 * D], f32, tag="e1")
                nmm = (KT + 1) // 2
                for t in range(nmm):
                    ks, ke = 2 * t, min(2 * t + 2, KT)
                    nc.tensor.matmul(
                        out=e1ps[:, 0:(ke - ks) * D],
                        lhsT=identg,
                        rhs=gt[:, ks:ke, 0:2 * D].bitcast(f16),
                        start=(t == 0), stop=(t == nmm - 1),
                    )
                # int8 [C2|C3] sections -> fp16 (scale folded into the convert)
                gc = vpool.tile([P, KT, 2 * D], f16, tag="gc")
                nc.vector.tensor_scalar(
                    out=gc[:], in0=gt[:, :, 2 * D:ROWB], scalar1=float(S8),
                    scalar2=None, op0=Alu.mult,
                )
                e23ps = ps_e23.tile([P, 4 * D], f32, tag="e23")
                for t in range(nmm):
                    ks, ke = 2 * t, min(2 * t + 2, KT)
                    nc.tensor.matmul(
                        out=e23ps[:, 0:(ke - ks) * 2 * D],
                        lhsT=identg,
                        rhs=gc[:, ks:ke, :],
                        start=(t == 0), stop=(t == nmm - 1),
                    )
                # combine the two fused-slot sections; only one tensor input
                # may read PSUM, so stage one section through SBUF on ACT
                eb = g * 3 * D
                t1 = wpool.tile([P, D], f32, tag="sec1")
                nc.scalar.copy(out=t1[:], in_=e1ps[:, D:2 * D])
                nc.vector.tensor_tensor(
                    out=E_all[:, eb:eb + D], in0=e1ps[:, 0:D],
                    in1=t1[:], op=Alu.add,
                )
                t2 = wpool.tile([P, 2 * D], f32, tag="sec23")
                nc.scalar.copy(out=t2[:], in_=e23ps[:, 2 * D:4 * D])
                nc.vector.tensor_tensor(
                    out=E_all[:, eb + D:eb + 3 * D], in0=e23ps[:, 0:2 * D],
                    in1=t2[:], op=Alu.add,
                )

                # F1/F2: E1^T and E2^T column blocks for the logits matmuls
                for t, F in ((0, F1), (1, F2)):
                    tp = ps_t.tile([P, P], f32, tag="tp")
                    nc.tensor.transpose(
                        out=tp[:], in_=E_all[:, eb + t * D:eb + (t + 1) * D],
                        identity=ident[:],
                    )
                    nc.scalar.copy(out=F[:, g * P:(g + 1) * P], in_=tp[:])
                # hop 0: u = mean_M E1 (softmax of zero logits is uniform)
                nc.tensor.matmul(
                    out=us[:], lhsT=sel[:, g * 2:(g + 1) * 2],
                    rhs=E_all[:, eb:eb + 3 * D],
                    start=(g == 0), stop=(g == NG - 1),
                )

            u = wpool.tile([BL, D], f32, tag="u0")
            nc.scalar.activation(
                out=u[:], in_=us[0:BL, 0:D], func=Act.Copy, scale=1.0 / M
            )

            # ---- hops 1..2
            for hop in (1, 2):
                F = F1 if hop == 1 else F2
                uc_ps = ps_col.tile([P, BL], f32, tag="colT")
                nc.tensor.matmul(out=uc_ps[:], lhsT=u[:], rhs=i2[:], start=True, stop=True)
                uc = wpool.tile([P, BL], f32, tag="uc")
                nc.scalar.copy(out=uc[:], in_=uc_ps[:])
                lg_ps = ps_mm.tile([BL, NS], f32, tag="lg")
                nc.tensor.matmul(out=lg_ps[:], lhsT=uc[:], rhs=F[:], start=True, stop=True)
                lgm = wpool.tile([BL, NS], f32, tag="lgm")
                nc.vector.scalar_tensor_tensor(
                    out=lgm[:], in0=lg_ps[:], scalar=1.0, in1=mneg[:],
                    op0=Alu.mult, op1=Alu.add,
                )
                nmax = wpool.tile([BL, 1], f32, tag="nmax")
                nc.vector.tensor_reduce(
                    out=nmax[:], in_=lgm[:], axis=Ax.X, op=Alu.max, negate=True
                )
                pe = wpool.tile([BL, NS], f32, tag="pe")
                den = wpool.tile([BL, 1], f32, tag="den")
                nc.scalar.activation(
                    out=pe[:], in_=lgm[:], func=Act.Exp, bias=nmax[:], scale=1.0,
                    accum_out=den[:],
                )
                rden = wpool.tile([BL, 1], f32, tag="rden")
                nc.vector.reciprocal(out=rden[:], in_=den[:])
                # prob columns: ptall[:, 2g:2g+2] = pe[:, gP:(g+1)P]^T
                ptall = wpool.tile([P, NG * BL], f32, tag="ptall")
                for g in range(NG):
                    pt_ps = ps_col.tile([P, BL], f32, tag="colT")
                    nc.tensor.matmul(
                        out=pt_ps[:], lhsT=pe[:, g * P:(g + 1) * P], rhs=i2[:],
                        start=True, stop=True,
                    )
                    nc.scalar.copy(out=ptall[:, g * BL:(g + 1) * BL], in_=pt_ps[:])
                o_ps = ps_mm.tile([BL, D], f32, tag="o")
                for g in range(NG):
                    nc.tensor.matmul(
                        out=o_ps[:], lhsT=ptall[:, g * BL:(g + 1) * BL],
                        rhs=E_all[:, g * 3 * D + hop * D: g * 3 * D + hop * D + D],
                        start=(g == 0), stop=(g == NG - 1),
                    )
                u2 = wpool.tile([BL, D], f32, tag=f"u{hop}")
                nc.vector.scalar_tensor_tensor(
                    out=u2[:], in0=o_ps[:], scalar=rden[:], in1=u[:],
                    op0=Alu.mult, op1=Alu.add,
                )
                u = u2

            nc.sync.dma_start(out=out_d[:], in_=u[:])
    if do_compile:
        nc.compile()
    return nc


def _wrap16(idx):
    """flat [n] int16 -> SBUF layout [128, n//16]: value i at [i%16, i//16],
    replicated to the 8 16-partition groups the Q7 cores read."""
    n = idx.shape[0]
    w = np.zeros((16, n // 16), np.int16)
    w[np.arange(n) % 16, np.arange(n) // 16] = idx
    return np.tile(w, (8, 1))


def prep_inputs(story, C):
    """Host-side: fused fp16/int8 table + balanced exact-count index layouts."""
    story = np.asarray(story)
    C = np.asarray(C, dtype=np.float32)
    s = story.transpose(1, 0, 2).astype(np.int32)       # (B, M, S)

    ccat = np.zeros((V + 1, ROWB), np.int8)
    ccat[:V, 0:256] = C[1].astype(np.float16).view(np.int8)
    for j, t in ((2, 2), (3, 3)):
        q = np.clip(np.round(C[t] / S8), -127, 127).astype(np.int8)
        ccat[:V, 128 * (j):128 * (j + 1)] = q

    # per core: sort tokens per sentence, group sentences by nmin quartile;
    # pick per-group split KA so (almost) every sentence sends exactly KA
    # tokens to call A and 64-KA to call B
    per_core = []
    for i in range(NCORES):
        blk = np.sort(s[i * BL:(i + 1) * BL].reshape(NS, S), axis=1)
        own = np.repeat(np.arange(BL), M)
        nmin = (blk < VB).sum(1)
        nmax = (blk < IMAX).sum(1)
        order = np.argsort(nmin, kind="stable")
        groups = []
        for g in range(NG):
            pick = order[g * P:(g + 1) * P]
            groups.append((blk[pick], nmin[pick], nmax[pick], own[pick]))
        per_core.append(groups)

    KA = tuple(
        int(max(per_core[i][g][1].max() for i in range(NCORES)))
        for g in range(NG)
    )
    # per-sentence A-count (clamped for rare infeasible sentences)
    KB = []
    for g in range(NG):
        kb = 64 - KA[g]
        for i in range(NCORES):
            _, nmin, nmax, _ = per_core[i][g]
            a = np.clip(KA[g], nmin, nmax)
            kb = max(kb, int(64 - a.min()))
        KB.append(kb)
    KB = tuple(KB)

    ident = np.eye(P, dtype=np.float32)
    identg = np.eye(P, dtype=np.float16)
    i2 = np.eye(2, dtype=np.float32)
    offs, NB = _blob_cols(KA, KB)

    in_maps = []
    for i in range(NCORES):
        sel = np.zeros((P, NG * 2), np.float32)
        mneg = np.full((BL, BL * M), NEG, np.float32)
        blob = np.zeros((P, NB), np.int16)
        blob[:, offs["identg"]:offs["identg"] + D] = identg.view(np.int16)
        for g in range(NG):
            toks, nmin, nmax, owner = per_core[i][g]
            sel[np.arange(P), g * 2 + owner] = 1.0
            mneg[owner, g * P + np.arange(P)] = 0.0
            a = np.clip(KA[g], nmin, nmax)                  # (P,)
            ks = np.arange(KA[g])[:, None]                  # slot k, sentence p
            low = np.where(ks < a[None, :], toks.T[:KA[g]], 0).astype(np.int16)
            blob[:, offs[g, "a"]:offs[g, "a"] + 8 * KA[g]] = _wrap16(
                low.reshape(-1))
            # B slot j of sentence p holds token a[p]+j (shifted), else pad
            js = np.arange(KB[g])[:, None]                  # (KB, 1)
            src = np.minimum(a[None, :] + js, S - 1)
            high = np.where(
                js < (S - a)[None, :],
                np.take_along_axis(toks.T, src, axis=0).astype(np.int64) - VB,
                ZB,
            ).astype(np.int16)
            blob[:, offs[g, "b"]:offs[g, "b"] + 8 * KB[g]] = _wrap16(
                high.reshape(-1))
        in_maps.append({
            "ccat": ccat, "blob": blob, "ident": ident, "sel": sel,
            "i2": i2, "mneg": mneg,
        })
    return in_maps, KA, KB


def run(in_maps, KA, KB, trace=False, **kwargs):
    from concourse.bass_utils import run_bass_kernel_spmd

    key = (KA, KB)
    if key not in _CACHE:
        _CACHE[key] = build(KA, KB)
    nc = _CACHE[key]
    res = run_bass_kernel_spmd(
        nc, in_maps, core_ids=list(range(NCORES)), trace=trace, **kwargs
    )
    out = np.concatenate([r["out"] for r in res.results], axis=0)
    return out, res


def kernel(story, C):
    in_maps, KA, KB = prep_inputs(story, C)
    out, _ = run(in_maps, KA, KB)
    return out.astype(np.float32)


# revision 14
# speedup vs baseline: 1.5138x; 1.0216x over previous
"""Trainium2 Bass kernel for nn_EncoderMemNN_14929306321427 (MemNN encoder).

Math (see reference.py): story (M=256, B=16, S=64) token ids; C (4, V, 128)
embedding tables. Per hop h: m_A = sum_S C[h][s], prob = softmax_M(m_A @ u),
m_C = sum_S C[h+1][s], u += prob @ m_C. u starts at 0, so hop-0's softmax is
uniform: C[0] is never needed and u after hop 0 is mean_M(E1).

Strategy: data-parallel over batch (2 rows/core, 8 cores, no collectives).
Host fuses tables 1..3 into ccat[V+1, 512B] rows: [C1 fp16 | C2 int8 | C3
int8] (row V = 0). Each token is ONE 512B dma_gather row. dma_gather indices
are int16, so tokens split between call A (table base, t<32768) and call B
(base row VB=17490, t>=VB); the overlap zone lets the host pick a per-group
split point KA so every sentence contributes EXACTLY KA rows to A and 64-KA
to B (zero-row padding only for rare infeasible sentences). Each group's 64
slots are cut into 4 equal pieces issued on the 4 SWDGE queues -- descriptor
generation (~8 ns/idx/queue, the critical resource) runs 4-way parallel
across the 8 Q7 cores. The sentence-sum runs on the PE as identity-matmul
accumulation into PSUM (2 slots fused per matmul); the int8 sections are
converted+scaled to fp16 on the DVE first. A small PE/ACT/DVE attention
pipeline computes the 3 hops.
"""

import numpy as np

HOPS = 3
V = 50257
D = 128
M = 256
B = 16
S = 64
NCORES = 8
BL = B // NCORES            # batch rows per core
NS = BL * M                 # sentences per core
P = 128
NG = NS // P                # sentence groups of 128
ROWB = 512                  # fused row bytes: 256 (C1 f16) + 128 + 128 (int8)
NEG = -1e30
IMAX = 32768                # call A covers idx 0..32767
VB = V + 1 - IMAX           # 17490: call B covers rows VB..V (V = zero row)
ZB = V - VB                 # 32767: B-call index of the zero row
CLIP = 0.4                  # int8 clip (~4 sigma of N(0, 0.1))
S8 = CLIP / 127.0

_CACHE = {}


def _pieces(KA_g, KB_g):
    """Cut a group's KT slots into 4 near-equal queue pieces; a piece
    straddling the A/B boundary becomes two calls on the same queue. The
    queue-0 call blocks the GpSimd engine until generated, so issue it last."""
    KT = KA_g + KB_g
    sizes = [KT // 4 + (1 if i < KT % 4 else 0) for i in range(4)]
    cuts = np.cumsum([0] + sizes)
    specs = []
    for q in range(4):
        k0, k1 = int(cuts[q]), int(cuts[q + 1])
        if k1 <= KA_g:
            specs.append((q, k0, k1, True))
        elif k0 >= KA_g:
            specs.append((q, k0, k1, False))
        else:
            specs.append((q, k0, KA_g, True))
            specs.append((q, KA_g, k1, False))
    return [s for s in specs if s[0] != 0] + [s for s in specs if s[0] == 0]


def _blob_cols(KA, KB):
    """Column offsets of per-group idx sections + identg inside the blob."""
    offs, c = {}, 0
    for g in range(NG):
        offs[g, "a"] = c
        c += 8 * KA[g]
        offs[g, "b"] = c
        c += 8 * KB[g]
    offs["identg"] = c
    c += D
    return offs, c


def build(KA, KB, do_compile=True):
    from concourse import bacc, mybir, tile

    f32 = mybir.dt.float32
    f16 = mybir.dt.float16
    i16 = mybir.dt.int16
    i8 = mybir.dt.int8
    Alu = mybir.AluOpType
    Act = mybir.ActivationFunctionType
    Ax = mybir.AxisListType

    offs, NB = _blob_cols(KA, KB)

    nc = bacc.Bacc(num_swdge_queues=4)
    ccat_d = nc.declare_dram_parameter("ccat", [V + 1, ROWB], i8, isOutput=False)
    blob_d = nc.declare_dram_parameter("blob", [P, NB], i16, isOutput=False)
    ident_d = nc.declare_dram_parameter("ident", [P, P], f32, isOutput=False)
    sel_d = nc.declare_dram_parameter("sel", [P, NG * 2], f32, isOutput=False)
    i2_d = nc.declare_dram_parameter("i2", [2, 2], f32, isOutput=False)
    mneg_d = nc.declare_dram_parameter("mneg", [BL, BL * M], f32, isOutput=False)
    out_d = nc.declare_dram_parameter("out", [BL, D], f32, isOutput=True)

    with tile.TileContext(nc) as tc:
        with (
            tc.tile_pool(name="const", bufs=1) as cpool,
            tc.tile_pool(name="gather", bufs=3) as gpool,
            tc.tile_pool(name="conv", bufs=2) as vpool,
            tc.tile_pool(name="work", bufs=2) as wpool,
            tc.tile_pool(name="ps_e1", bufs=1, space="PSUM") as ps_e1,
            tc.tile_pool(name="ps_e23", bufs=2, space="PSUM") as ps_e23,
            tc.tile_pool(name="ps_t", bufs=2, space="PSUM") as ps_t,
            tc.tile_pool(name="ps_col", bufs=2, space="PSUM") as ps_col,
            tc.tile_pool(name="ps_mm", bufs=1, space="PSUM") as ps_mm,
        ):
            # dummy gather first: pays the GPSIMD ext-isa library load +
            # SWDGE ring setup while the idx blob is still streaming in
            dummy_i = cpool.tile([P, 8], i16)
            nc.vector.memset(dummy_i[:], 0)
            dummy_o = cpool.tile([P, 1, ROWB], i8)
            nc.gpsimd.dma_gather(
                out_ap=dummy_o[:], in_ap=ccat_d[:], idxs_ap=dummy_i[:],
                num_idxs=P, num_idxs_reg=P, elem_size=ROWB,
                single_packet=False, queue_num=0,
            )
            # idx blob -- the first real gather only waits on this load
            blob = cpool.tile([P, NB], i16)
            nc.sync.dma_start(out=blob[:], in_=blob_d[:])
            identg = blob[:, offs["identg"]:offs["identg"] + D].bitcast(f16)
            ident = cpool.tile([P, P], f32)
            nc.sync.dma_start(out=ident[:], in_=ident_d[:])
            sel = cpool.tile([P, NG * 2], f32)
            nc.sync.dma_start(out=sel[:], in_=sel_d[:])
            i2 = cpool.tile([2, 2], f32)
            nc.sync.dma_start(out=i2[:], in_=i2_d[:])
            mneg = cpool.tile([BL, BL * M], f32)
            nc.sync.dma_start(out=mneg[:], in_=mneg_d[:])

            # E_all[p, g*384+d]: per-sentence sums [E1|E2|E3] for group g
            E_all = cpool.tile([P, NG * 3 * D], f32)
            F1 = cpool.tile([P, NS], f32)
            F2 = cpool.tile([P, NS], f32)
            # us/lg/o are alive at disjoint times -- share one PSUM bank
            us = ps_mm.tile([BL, NS], f32, tag="mm")

            for g in range(NG):
                KT = KA[g] + KB[g]
                gt = gpool.tile([P, KT, ROWB], i8, tag="gt")
                for q, k0, k1, is_a in _pieces(KA[g], KB[g]):
                    if is_a:
                        in_ap = ccat_d[:]
                        c0 = offs[g, "a"] + 8 * k0
                        c1 = offs[g, "a"] + 8 * k1
                    else:
                        in_ap = ccat_d[VB:, :]
                        c0 = offs[g, "b"] + 8 * (k0 - KA[g])
                        c1 = offs[g, "b"] + 8 * (k1 - KA[g])
                    nc.gpsimd.dma_gather(
                        out_ap=gt[:, k0:k1, :], in_ap=in_ap,
                        idxs_ap=blob[:, c0:c1],
                        num_idxs=P * (k1 - k0), num_idxs_reg=P * (k1 - k0),
                        elem_size=ROWB, single_packet=False, queue_num=q,
                    )

                # E1: identity-matmul accumulation of the fp16 C1 sections,
                # 2 slots fused per matmul (sections recombined on DVE)
                e1ps = ps_e1.tile([P, 2 * D], f32, tag="e1")
                nmm = (KT + 1) // 2
                for t in range(nmm):
                    ks, ke = 2 * t, min(2 * t + 2, KT)
                    nc.tensor.matmul(
                        out=e1ps[:, 0:(ke - ks) * D],
                        lhsT=identg,
                        rhs=gt[:, ks:ke, 0:2 * D].bitcast(f16),
                        start=(t == 0), stop=(t == nmm - 1),
                    )
                # int8 [C2|C3] sections -> fp16 (scale folded into the
                # convert); split across DVE and ACT, which run concurrently
                gc = vpool.tile([P, KT, 2 * D], f16, tag="gc")
                kh = KT // 2
                nc.vector.tensor_scalar(
                    out=gc[:, 0:kh, :], in0=gt[:, 0:kh, 2 * D:ROWB],
                    scalar1=float(S8), scalar2=None, op0=Alu.mult,
                )
                nc.scalar.activation(
                    out=gc[:, kh:KT, :], in_=gt[:, kh:KT, 2 * D:ROWB],
                    func=Act.Copy, scale=float(S8),
                )
                e23ps = ps_e23.tile([P, 4 * D], f32, tag="e23")
                for t in range(nmm):
                    ks, ke = 2 * t, min(2 * t + 2, KT)
                    nc.tensor.matmul(
                        out=e23ps[:, 0:(ke - ks) * 2 * D],
                        lhsT=identg,
                        rhs=gc[:, ks:ke, :],
                        start=(t == 0), stop=(t == nmm - 1),
                    )
                # combine the two fused-slot sections; only one tensor input
                # may read PSUM, so stage one section through SBUF on ACT
                eb = g * 3 * D
                t1 = wpool.tile([P, D], f32, tag="sec1")
                nc.scalar.copy(out=t1[:], in_=e1ps[:, D:2 * D])
                nc.vector.tensor_tensor(
                    out=E_all[:, eb:eb + D], in0=e1ps[:, 0:D],
                    in1=t1[:], op=Alu.add,
                )
                t2 = wpool.tile([P, 2 * D], f32, tag="sec23")
                nc.scalar.copy(out=t2[:], in_=e23ps[:, 2 * D:4 * D])
                nc.vector.tensor_tensor(
                    out=E_all[:, eb + D:eb + 3 * D], in0=e23ps[:, 0:2 * D],
                    in1=t2[:], op=Alu.add,
                )

                # F1/F2: E1^T and E2^T column blocks for the logits matmuls
                for t, F in ((0, F1), (1, F2)):
                    tp = ps_t.tile([P, P], f32, tag="tp")
                    nc.tensor.transpose(
                        out=tp[:], in_=E_all[:, eb + t * D:eb + (t + 1) * D],
                        identity=ident[:],
                    )
                    nc.scalar.copy(out=F[:, g * P:(g + 1) * P], in_=tp[:])
                # hop 0: u = mean_M E1 (softmax of zero logits is uniform)
                nc.tensor.matmul(
                    out=us[0:BL, 0:3 * D], lhsT=sel[:, g * 2:(g + 1) * 2],
                    rhs=E_all[:, eb:eb + 3 * D],
                    start=(g == 0), stop=(g == NG - 1),
                )

            u = wpool.tile([BL, D], f32, tag="u0")
            nc.scalar.activation(
                out=u[:], in_=us[0:BL, 0:D], func=Act.Copy, scale=1.0 / M
            )

            # ---- hops 1..2
            for hop in (1, 2):
                F = F1 if hop == 1 else F2
                uc_ps = ps_col.tile([P, BL], f32, tag="colT")
                nc.tensor.matmul(out=uc_ps[:], lhsT=u[:], rhs=i2[:], start=True, stop=True)
                uc = wpool.tile([P, BL], f32, tag="uc")
                nc.scalar.copy(out=uc[:], in_=uc_ps[:])
                lg_ps = ps_mm.tile([BL, NS], f32, tag="mm")
                nc.tensor.matmul(out=lg_ps[:], lhsT=uc[:], rhs=F[:], start=True, stop=True)
                lgm = wpool.tile([BL, NS], f32, tag="lgm")
                nc.vector.scalar_tensor_tensor(
                    out=lgm[:], in0=lg_ps[:], scalar=1.0, in1=mneg[:],
                    op0=Alu.mult, op1=Alu.add,
                )
                nmax = wpool.tile([BL, 1], f32, tag="nmax")
                nc.vector.tensor_reduce(
                    out=nmax[:], in_=lgm[:], axis=Ax.X, op=Alu.max, negate=True
                )
                pe = wpool.tile([BL, NS], f32, tag="pe")
                den = wpool.tile([BL, 1], f32, tag="den")
                nc.scalar.activation(
                    out=pe[:], in_=lgm[:], func=Act.Exp, bias=nmax[:], scale=1.0,
                    accum_out=den[:],
                )
                rden = wpool.tile([BL, 1], f32, tag="rden")
                nc.vector.reciprocal(out=rden[:], in_=den[:])
                # prob columns: ptall[:, 2g:2g+2] = pe[:, gP:(g+1)P]^T
                ptall = wpool.tile([P, NG * BL], f32, tag="ptall")
                for g in range(NG):
                    pt_ps = ps_col.tile([P, BL], f32, tag="colT")
                    nc.tensor.matmul(
                        out=pt_ps[:], lhsT=pe[:, g * P:(g + 1) * P], rhs=i2[:],
                        start=True, stop=True,
                    )
                    nc.scalar.copy(out=ptall[:, g * BL:(g + 1) * BL], in_=pt_ps[:])
                o_ps = ps_mm.tile([BL, NS], f32, tag="mm")
                for g in range(NG):
                    nc.tensor.matmul(
                        out=o_ps[0:BL, 0:D], lhsT=ptall[:, g * BL:(g + 1) * BL],
                        rhs=E_all[:, g * 3 * D + hop * D: g * 3 * D + hop * D + D],
                        start=(g == 0), stop=(g == NG - 1),
                    )
                u2 = wpool.tile([BL, D], f32, tag=f"u{hop}")
                nc.vector.scalar_tensor_tensor(
                    out=u2[:], in0=o_ps[0:BL, 0:D], scalar=rden[:], in1=u[:],
                    op0=Alu.mult, op1=Alu.add,
                )
                u = u2

            nc.sync.dma_start(out=out_d[:], in_=u[:])
    if do_compile:
        nc.compile()
    return nc


def _wrap16(idx):
    """flat [n] int16 -> SBUF layout [128, n//16]: value i at [i%16, i//16],
    replicated to the 8 16-partition groups the Q7 cores read."""
    n = idx.shape[0]
    w = np.zeros((16, n // 16), np.int16)
    w[np.arange(n) % 16, np.arange(n) // 16] = idx
    return np.tile(w, (8, 1))


def prep_inputs(story, C):
    """Host-side: fused fp16/int8 table + balanced exact-count index layouts."""
    story = np.asarray(story)
    C = np.asarray(C, dtype=np.float32)
    s = story.transpose(1, 0, 2).astype(np.int32)       # (B, M, S)

    ccat = np.zeros((V + 1, ROWB), np.int8)
    ccat[:V, 0:256] = C[1].astype(np.float16).view(np.int8)
    for j, t in ((2, 2), (3, 3)):
        q = np.clip(np.round(C[t] / S8), -127, 127).astype(np.int8)
        ccat[:V, 128 * (j):128 * (j + 1)] = q

    # per core: sort tokens per sentence, group sentences by nmin quartile;
    # pick per-group split KA so (almost) every sentence sends exactly KA
    # tokens to call A and 64-KA to call B
    per_core = []
    for i in range(NCORES):
        blk = np.sort(s[i * BL:(i + 1) * BL].reshape(NS, S), axis=1)
        own = np.repeat(np.arange(BL), M)
        nmin = (blk < VB).sum(1)
        nmax = (blk < IMAX).sum(1)
        order = np.argsort(nmin, kind="stable")
        groups = []
        for g in range(NG):
            pick = order[g * P:(g + 1) * P]
            groups.append((blk[pick], nmin[pick], nmax[pick], own[pick]))
        per_core.append(groups)

    KA = tuple(
        int(max(per_core[i][g][1].max() for i in range(NCORES)))
        for g in range(NG)
    )
    # per-sentence A-count (clamped for rare infeasible sentences)
    KB = []
    for g in range(NG):
        kb = 64 - KA[g]
        for i in range(NCORES):
            _, nmin, nmax, _ = per_core[i][g]
            a = np.clip(KA[g], nmin, nmax)
            kb = max(kb, int(64 - a.min()))
        KB.append(kb)
    KB = tuple(KB)

    ident = np.eye(P, dtype=np.float32)
    identg = np.eye(P, dtype=np.float16)
    i2 = np.eye(2, dtype=np.float32)
    offs, NB = _blob_cols(KA, KB)

    in_maps = []
    for i in range(NCORES):
        sel = np.zeros((P, NG * 2), np.float32)
        mneg = np.full((BL, BL * M), NEG, np.float32)
        blob = np.zeros((P, NB), np.int16)
        blob[:, offs["identg"]:offs["identg"] + D] = identg.view(np.int16)
        for g in range(NG):
            toks, nmin, nmax, owner = per_core[i][g]
            sel[np.arange(P), g * 2 + owner] = 1.0
            mneg[owner, g * P + np.arange(P)] = 0.0
            a = np.clip(KA[g], nmin, nmax)                  # (P,)
            ks = np.arange(KA[g])[:, None]                  # slot k, sentence p
            low = np.where(ks < a[None, :], toks.T[:KA[g]], 0).astype(np.int16)
            blob[:, offs[g, "a"]:offs[g, "a"] + 8 * KA[g]] = _wrap16(
                low.reshape(-1))
            # B slot j of sentence p holds token a[p]+j (shifted), else pad
            js = np.arange(KB[g])[:, None]                  # (KB, 1)
            src = np.minimum(a[None, :] + js, S - 1)
            high = np.where(
                js < (S - a)[None, :],
                np.take_along_axis(toks.T, src, axis=0).astype(np.int64) - VB,
                ZB,
            ).astype(np.int16)
            blob[:, offs[g, "b"]:offs[g, "b"] + 8 * KB[g]] = _wrap16(
                high.reshape(-1))
        in_maps.append({
            "ccat": ccat, "blob": blob, "ident": ident, "sel": sel,
            "i2": i2, "mneg": mneg,
        })
    return in_maps, KA, KB


def run(in_maps, KA, KB, trace=False, **kwargs):
    from concourse.bass_utils import run_bass_kernel_spmd

    key = (KA, KB)
    if key not in _CACHE:
        _CACHE[key] = build(KA, KB)
    nc = _CACHE[key]
    res = run_bass_kernel_spmd(
        nc, in_maps, core_ids=list(range(NCORES)), trace=trace, **kwargs
    )
    out = np.concatenate([r["out"] for r in res.results], axis=0)
    return out, res


def kernel(story, C):
    in_maps, KA, KB = prep_inputs(story, C)
    out, _ = run(in_maps, KA, KB)
    return out.astype(np.float32)


# revision 16
# speedup vs baseline: 1.6409x; 1.0840x over previous
"""Trainium2 Bass kernel for nn_EncoderMemNN_14929306321427 (MemNN encoder).

Math (see reference.py): story (M=256, B=16, S=64) token ids; C (4, V, 128)
embedding tables. Per hop h: m_A = sum_S C[h][s], prob = softmax_M(m_A @ u),
m_C = sum_S C[h+1][s], u += prob @ m_C. u starts at 0, so hop-0's softmax is
uniform: C[0] is never needed and u after hop 0 is mean_M(E1).

Strategy: data-parallel over batch (2 rows/core, 8 cores, no collectives).
Host fuses tables 1..3 into ccat[V+1, 384] fp16 rows [C1|C2|C3] (row V = 0);
each token is ONE 768B dma_gather row. dma_gather indices are int16, so
tokens split between call A (table base, t<32768) and call B (base row
VB=17490, t>=VB); the overlap zone lets the host pick a per-group split KA so
every sentence contributes EXACTLY KA rows to A and 64-KA to B (zero-row
padding only for rare infeasible sentences). Each group's 64 slots are cut
into 4 equal pieces issued on the 4 SWDGE queues -- descriptor generation
(~8 ns/idx/queue, one Q7 core pair per queue) runs 4-way parallel. The
sentence-sum runs on the PE as identity-matmul accumulation into PSUM, two
768-col slots fused per matmul; the two PSUM sections are recombined by
ACT-copy + DVE-add straight into the [E1|E2|E3] layout the attention needs.
A small PE/ACT/DVE attention pipeline computes the 3 hops.
"""

import numpy as np

HOPS = 3
V = 50257
D = 128
M = 256
B = 16
S = 64
NCORES = 8
BL = B // NCORES            # batch rows per core
NS = BL * M                 # sentences per core
P = 128
NG = NS // P                # sentence groups of 128
DCAT = HOPS * D             # 384 f16 elems per fused row (768 B)
NEG = -1e30
IMAX = 32768                # call A covers idx 0..32767
VB = V + 1 - IMAX           # 17490: call B covers rows VB..V (V = zero row)
ZB = V - VB                 # 32767: B-call index of the zero row

_CACHE = {}


def _pieces(KA_g, KB_g):
    """Cut a group's KT slots into 4 near-equal queue pieces; a piece
    straddling the A/B boundary becomes two calls on the same queue. The
    queue-0 call blocks the GpSimd engine until generated, so issue it last."""
    KT = KA_g + KB_g
    sizes = [KT // 4 + (1 if i < KT % 4 else 0) for i in range(4)]
    cuts = np.cumsum([0] + sizes)
    specs = []
    for q in range(4):
        k0, k1 = int(cuts[q]), int(cuts[q + 1])
        if k1 <= KA_g:
            specs.append((q, k0, k1, True))
        elif k0 >= KA_g:
            specs.append((q, k0, k1, False))
        else:
            specs.append((q, k0, KA_g, True))
            specs.append((q, KA_g, k1, False))
    return [s for s in specs if s[0] != 0] + [s for s in specs if s[0] == 0]


def _blob_cols(KA, KB):
    """Column offsets of per-group idx sections + identg inside the blob."""
    offs, c = {}, 0
    for g in range(NG):
        offs[g, "a"] = c
        c += 8 * KA[g]
        offs[g, "b"] = c
        c += 8 * KB[g]
    offs["identg"] = c
    c += D
    return offs, c


def build(KA, KB, do_compile=True):
    from concourse import bacc, mybir, tile

    f32 = mybir.dt.float32
    f16 = mybir.dt.float16
    i16 = mybir.dt.int16
    Alu = mybir.AluOpType
    Act = mybir.ActivationFunctionType
    Ax = mybir.AxisListType

    offs, NB = _blob_cols(KA, KB)

    nc = bacc.Bacc(num_swdge_queues=4)
    ccat_d = nc.declare_dram_parameter("ccat", [V + 1, DCAT], f16, isOutput=False)
    blob_d = nc.declare_dram_parameter("blob", [P, NB], i16, isOutput=False)
    ident_d = nc.declare_dram_parameter("ident", [P, P], f32, isOutput=False)
    sel_d = nc.declare_dram_parameter("sel", [P, NG * 2], f32, isOutput=False)
    i2_d = nc.declare_dram_parameter("i2", [2, 2], f32, isOutput=False)
    mneg_d = nc.declare_dram_parameter("mneg", [BL, BL * M], f32, isOutput=False)
    out_d = nc.declare_dram_parameter("out", [BL, D], f32, isOutput=True)

    with tile.TileContext(nc) as tc:
        with (
            tc.tile_pool(name="const", bufs=1) as cpool,
            tc.tile_pool(name="gather", bufs=3) as gpool,
            tc.tile_pool(name="work", bufs=2) as wpool,
            tc.tile_pool(name="ps_e", bufs=2, space="PSUM") as ps_e,
            tc.tile_pool(name="ps_t", bufs=1, space="PSUM") as ps_t,
            tc.tile_pool(name="ps_col", bufs=2, space="PSUM") as ps_col,
            tc.tile_pool(name="ps_mm", bufs=1, space="PSUM") as ps_mm,
        ):
            # dummy gather first: pays the GPSIMD ext-isa library load +
            # SWDGE ring setup while the idx blob is still streaming in
            dummy_i = cpool.tile([P, 8], i16)
            nc.vector.memset(dummy_i[:], 0)
            dummy_o = cpool.tile([P, 1, DCAT], f16)
            nc.gpsimd.dma_gather(
                out_ap=dummy_o[:], in_ap=ccat_d[:], idxs_ap=dummy_i[:],
                num_idxs=P, num_idxs_reg=P, elem_size=DCAT,
                single_packet=False, queue_num=0,
            )
            # idx blob -- the first real gather only waits on this load
            blob = cpool.tile([P, NB], i16)
            nc.sync.dma_start(out=blob[:], in_=blob_d[:])
            identg = blob[:, offs["identg"]:offs["identg"] + D].bitcast(f16)
            ident = cpool.tile([P, P], f32)
            nc.sync.dma_start(out=ident[:], in_=ident_d[:])
            sel = cpool.tile([P, NG * 2], f32)
            nc.sync.dma_start(out=sel[:], in_=sel_d[:])
            i2 = cpool.tile([2, 2], f32)
            nc.sync.dma_start(out=i2[:], in_=i2_d[:])
            mneg = cpool.tile([BL, BL * M], f32)
            nc.sync.dma_start(out=mneg[:], in_=mneg_d[:])

            # E_all[p, g*384+d]: per-sentence sums [E1|E2|E3] for group g
            E_all = cpool.tile([P, NG * DCAT], f32)
            F1 = cpool.tile([P, NS], f32)
            F2 = cpool.tile([P, NS], f32)
            # us/lg/o are alive at disjoint times -- share one PSUM bank
            us = ps_mm.tile([BL, NS], f32, tag="mm")

            for g in range(NG):
                KT = KA[g] + KB[g]
                gt = gpool.tile([P, KT, DCAT], f16, tag="gt")
                for q, k0, k1, is_a in _pieces(KA[g], KB[g]):
                    if is_a:
                        in_ap = ccat_d[:]
                        c0 = offs[g, "a"] + 8 * k0
                        c1 = offs[g, "a"] + 8 * k1
                    else:
                        in_ap = ccat_d[VB:, :]
                        c0 = offs[g, "b"] + 8 * (k0 - KA[g])
                        c1 = offs[g, "b"] + 8 * (k1 - KA[g])
                    nc.gpsimd.dma_gather(
                        out_ap=gt[:, k0:k1, :], in_ap=in_ap,
                        idxs_ap=blob[:, c0:c1],
                        num_idxs=P * (k1 - k0), num_idxs_reg=P * (k1 - k0),
                        elem_size=DCAT, single_packet=False, queue_num=q,
                    )

                # identity-matmul accumulation, 2 slots per matmul; a matmul
                # output must fit one PSUM bank (512 f32), so the 384-wide
                # rows are split into a [C1|C2] stream and a [C3] stream
                e12 = ps_e.tile([P, 4 * D], f32, tag="e12")
                e3 = ps_e.tile([P, 2 * D], f32, tag="e3")
                nmm = (KT + 1) // 2
                for t in range(nmm):
                    ks, ke = 2 * t, min(2 * t + 2, KT)
                    nc.tensor.matmul(
                        out=e12[:, 0:(ke - ks) * 2 * D],
                        lhsT=identg,
                        rhs=gt[:, ks:ke, 0:2 * D],
                        start=(t == 0), stop=(t == nmm - 1),
                    )
                for t in range(nmm):
                    ks, ke = 2 * t, min(2 * t + 2, KT)
                    nc.tensor.matmul(
                        out=e3[:, 0:(ke - ks) * D],
                        lhsT=identg,
                        rhs=gt[:, ks:ke, 2 * D:DCAT],
                        start=(t == 0), stop=(t == nmm - 1),
                    )
                # combine the two fused-slot sections; only one tensor input
                # may read PSUM, so stage one section through SBUF on ACT
                eb = g * DCAT
                t1 = wpool.tile([P, 2 * D], f32, tag="sec1")
                nc.scalar.copy(out=t1[:], in_=e12[:, 2 * D:4 * D])
                nc.vector.tensor_tensor(
                    out=E_all[:, eb:eb + 2 * D], in0=e12[:, 0:2 * D],
                    in1=t1[:], op=Alu.add,
                )
                t2 = wpool.tile([P, D], f32, tag="sec3")
                nc.scalar.copy(out=t2[:], in_=e3[:, D:2 * D])
                nc.vector.tensor_tensor(
                    out=E_all[:, eb + 2 * D:eb + DCAT], in0=e3[:, 0:D],
                    in1=t2[:], op=Alu.add,
                )

                # F1/F2: E1^T and E2^T column blocks for the logits matmuls
                for t, F in ((0, F1), (1, F2)):
                    tp = ps_t.tile([P, P], f32, tag="tp")
                    nc.tensor.transpose(
                        out=tp[:], in_=E_all[:, eb + t * D:eb + (t + 1) * D],
                        identity=ident[:],
                    )
                    nc.scalar.copy(out=F[:, g * P:(g + 1) * P], in_=tp[:])
                # hop 0: u = mean_M E1 (softmax of zero logits is uniform)
                nc.tensor.matmul(
                    out=us[0:BL, 0:DCAT], lhsT=sel[:, g * 2:(g + 1) * 2],
                    rhs=E_all[:, eb:eb + DCAT],
                    start=(g == 0), stop=(g == NG - 1),
                )

            u = wpool.tile([BL, D], f32, tag="u0")
            nc.scalar.activation(
                out=u[:], in_=us[0:BL, 0:D], func=Act.Copy, scale=1.0 / M
            )

            # ---- hops 1..2
            for hop in (1, 2):
                F = F1 if hop == 1 else F2
                uc_ps = ps_col.tile([P, BL], f32, tag="colT")
                nc.tensor.matmul(out=uc_ps[:], lhsT=u[:], rhs=i2[:], start=True, stop=True)
                uc = wpool.tile([P, BL], f32, tag="uc")
                nc.scalar.copy(out=uc[:], in_=uc_ps[:])
                lg_ps = ps_mm.tile([BL, NS], f32, tag="mm")
                nc.tensor.matmul(out=lg_ps[:], lhsT=uc[:], rhs=F[:], start=True, stop=True)
                lgm = wpool.tile([BL, NS], f32, tag="lgm")
                nc.vector.scalar_tensor_tensor(
                    out=lgm[:], in0=lg_ps[:], scalar=1.0, in1=mneg[:],
                    op0=Alu.mult, op1=Alu.add,
                )
                nmax = wpool.tile([BL, 1], f32, tag="nmax")
                nc.vector.tensor_reduce(
                    out=nmax[:], in_=lgm[:], axis=Ax.X, op=Alu.max, negate=True
                )
                pe = wpool.tile([BL, NS], f32, tag="pe")
                den = wpool.tile([BL, 1], f32, tag="den")
                nc.scalar.activation(
                    out=pe[:], in_=lgm[:], func=Act.Exp, bias=nmax[:], scale=1.0,
                    accum_out=den[:],
                )
                rden = wpool.tile([BL, 1], f32, tag="rden")
                nc.vector.reciprocal(out=rden[:], in_=den[:])
                # prob columns: ptall[:, 2g:2g+2] = pe[:, gP:(g+1)P]^T
                ptall = wpool.tile([P, NG * BL], f32, tag="ptall")
                for g in range(NG):
                    pt_ps = ps_col.tile([P, BL], f32, tag="colT")
                    nc.tensor.matmul(
                        out=pt_ps[:], lhsT=pe[:, g * P:(g + 1) * P], rhs=i2[:],
                        start=True, stop=True,
                    )
                    nc.scalar.copy(out=ptall[:, g * BL:(g + 1) * BL], in_=pt_ps[:])
                o_ps = ps_mm.tile([BL, NS], f32, tag="mm")
                for g in range(NG):
                    nc.tensor.matmul(
                        out=o_ps[0:BL, 0:D], lhsT=ptall[:, g * BL:(g + 1) * BL],
                        rhs=E_all[:, g * DCAT + hop * D: g * DCAT + hop * D + D],
                        start=(g == 0), stop=(g == NG - 1),
                    )
                u2 = wpool.tile([BL, D], f32, tag=f"u{hop}")
                nc.vector.scalar_tensor_tensor(
                    out=u2[:], in0=o_ps[0:BL, 0:D], scalar=rden[:], in1=u[:],
                    op0=Alu.mult, op1=Alu.add,
                )
                u = u2

            nc.sync.dma_start(out=out_d[:], in_=u[:])
    if do_compile:
        nc.compile()
    return nc


def _wrap16(idx):
    """flat [n] int16 -> SBUF layout [128, n//16]: value i at [i%16, i//16],
    replicated to the 8 16-partition groups the Q7 cores read."""
    n = idx.shape[0]
    w = np.zeros((16, n // 16), np.int16)
    w[np.arange(n) % 16, np.arange(n) // 16] = idx
    return np.tile(w, (8, 1))


def prep_inputs(story, C):
    """Host-side: fused fp16 table + balanced exact-count index layouts."""
    story = np.asarray(story)
    C = np.asarray(C, dtype=np.float32)
    s = story.transpose(1, 0, 2).astype(np.int32)       # (B, M, S)

    ccat = np.zeros((V + 1, DCAT), np.float16)
    ccat[:V] = np.concatenate([C[1], C[2], C[3]], axis=1).astype(np.float16)

    # per core: sort tokens per sentence, group sentences by nmin quartile;
    # pick per-group split KA so (almost) every sentence sends exactly KA
    # tokens to call A and 64-KA to call B
    per_core = []
    for i in range(NCORES):
        blk = np.sort(s[i * BL:(i + 1) * BL].reshape(NS, S), axis=1)
        own = np.repeat(np.arange(BL), M)
        nmin = (blk < VB).sum(1)
        nmax = (blk < IMAX).sum(1)
        order = np.argsort(nmin, kind="stable")
        groups = []
        for g in range(NG):
            pick = order[g * P:(g + 1) * P]
            groups.append((blk[pick], nmin[pick], nmax[pick], own[pick]))
        per_core.append(groups)

    KA = tuple(
        int(max(per_core[i][g][1].max() for i in range(NCORES)))
        for g in range(NG)
    )
    KB = []
    for g in range(NG):
        kb = 64 - KA[g]
        for i in range(NCORES):
            _, nmin, nmax, _ = per_core[i][g]
            a = np.clip(KA[g], nmin, nmax)
            kb = max(kb, int(64 - a.min()))
        KB.append(kb)
    KB = tuple(KB)

    ident = np.eye(P, dtype=np.float32)
    identg = np.eye(P, dtype=np.float16)
    i2 = np.eye(2, dtype=np.float32)
    offs, NB = _blob_cols(KA, KB)

    in_maps = []
    for i in range(NCORES):
        sel = np.zeros((P, NG * 2), np.float32)
        mneg = np.full((BL, BL * M), NEG, np.float32)
        blob = np.zeros((P, NB), np.int16)
        blob[:, offs["identg"]:offs["identg"] + D] = identg.view(np.int16)
        for g in range(NG):
            toks, nmin, nmax, owner = per_core[i][g]
            sel[np.arange(P), g * 2 + owner] = 1.0
            mneg[owner, g * P + np.arange(P)] = 0.0
            a = np.clip(KA[g], nmin, nmax)                  # (P,)
            ks = np.arange(KA[g])[:, None]                  # slot k, sentence p
            low = np.where(ks < a[None, :], toks.T[:KA[g]], 0).astype(np.int16)
            blob[:, offs[g, "a"]:offs[g, "a"] + 8 * KA[g]] = _wrap16(
                low.reshape(-1))
            # B slot j of sentence p holds token a[p]+j (shifted), else pad
            js = np.arange(KB[g])[:, None]                  # (KB, 1)
            src = np.minimum(a[None, :] + js, S - 1)
            high = np.where(
                js < (S - a)[None, :],
                np.take_along_axis(toks.T, src, axis=0).astype(np.int64) - VB,
                ZB,
            ).astype(np.int16)
            blob[:, offs[g, "b"]:offs[g, "b"] + 8 * KB[g]] = _wrap16(
                high.reshape(-1))
        in_maps.append({
            "ccat": ccat, "blob": blob, "ident": ident, "sel": sel,
            "i2": i2, "mneg": mneg,
        })
    return in_maps, KA, KB


def run(in_maps, KA, KB, trace=False, **kwargs):
    from concourse.bass_utils import run_bass_kernel_spmd

    key = (KA, KB)
    if key not in _CACHE:
        _CACHE[key] = build(KA, KB)
    nc = _CACHE[key]
    res = run_bass_kernel_spmd(
        nc, in_maps, core_ids=list(range(NCORES)), trace=trace, **kwargs
    )
    out = np.concatenate([r["out"] for r in res.results], axis=0)
    return out, res


def kernel(story, C):
    in_maps, KA, KB = prep_inputs(story, C)
    out, _ = run(in_maps, KA, KB)
    return out.astype(np.float32)
